# revision 25
# baseline (speedup 1.0000x reference)
"""fp8-everywhere rewrite: DoubleRow fp8 z1/z2 for ALL steps (context and
prediction), bf16 fc path, fp8 x1 feedback via engine copies (no DMA on the
critical path).

Layouts (per core, Bc=4096, chunks of NC=512, feature-major [feat, batch]):
- xz1 [128, 3*Bc] fp8: b0 = h1 rows 0:128; b1 = [h1hi(0:72); x1(72:120);
  ones(120)]; b2 = [h1hi(0:72); act(72:78); state(78:84); ones(84)].
- xh2 [128, 2*Bc] fp8: b0 = h2 rows 0:128; b1 rows 0:72 = h2hi.
- h2A [128, Bc], h2hi [73, Bc] bf16 (for the bf16 fc path, maintained for
  steps t >= cf-1 only).
- c1, c2 [128, 2*Bc] bf16: cols 0:Bc = rows 0:128, Bc:2Bc = rows 128:200
  (partitions 0:72).
- x1t ping-pong [48, Bc] bf16: fc2 output (= next step's x1).
- gates g per chunk [128, 4096] bf16, gate order i,f,o,g; PSUM slabs
  s3l [128,1536], s3h [72,1536] (i,f,o), s1l/s1h (g).
"""
import sys as _sys
for _p in ("/opt/trn_rl_repo", "/root/.axon_site/_ro/trn_rl_repo"):
    if _p not in _sys.path:
        _sys.path.append(_p)

import numpy as np
import ml_dtypes
import contextlib

import concourse.bass as bass
import concourse.tile as tile
from concourse import bacc, mybir

F = mybir.ActivationFunctionType
A = mybir.AluOpType
BF = mybir.dt.bfloat16
F32 = mybir.dt.float32
E4 = mybir.dt.float8e4
DR = mybir.MatmulPerfMode.DoubleRow

H, DT, DA = 200, 48, 6
HLO, HHI = 128, 72
NC = 512


def perm_indices():
    # gate order i,f,o,g; each gate split lo(128)+hi(72)
    gr = {"i": 0, "f": 200, "o": 600, "g": 400}
    order = ["i", "f", "o", "g"]
    lo = np.concatenate([np.arange(gr[g], gr[g] + HLO) for g in order])
    hi = np.concatenate([np.arange(gr[g] + HLO, gr[g] + H) for g in order])
    return np.concatenate([lo, hi])


def prep_weights(inp):
    P = perm_indices()
    f32 = np.float32
    Wih1, Whh1 = f32(inp["Wih1"]), f32(inp["Whh1"])
    Wih2, Whh2 = f32(inp["Wih2"]), f32(inp["Whh2"])
    W1, W2 = f32(inp["W1"]), f32(inp["W2"])
    b1, b2 = f32(inp["b1"]), f32(inp["b2"])
    bb1 = f32(inp["bih1"]) + f32(inp["bhh1"])
    bb2 = f32(inp["bih2"]) + f32(inp["bhh2"])

    Wt = Wih2[:, H:H + DT]
    Wact = Wt[:, 0:6] + Wt[:, 12:18] + Wt[:, 24:30] + Wt[:, 36:42]
    Wsta = Wt[:, 6:12] + Wt[:, 18:24] + Wt[:, 30:36] + Wt[:, 42:48]

    P_ = perm_indices()
    # bf16 pieces for prediction-phase accuracy: x1+bias for z1, acts+bias
    # for z2 (fp8 for these is what blows the error budget)
    wx1 = np.ascontiguousarray(
        np.concatenate([Wih1, bb1[:, None]], 1)[P_].T)  # [49, 800]
    wzs = np.ascontiguousarray(
        np.concatenate([Wact, Wsta, bb2[:, None]], 1)[P_].T)  # [13, 800]

    # fc1: lp=[h2(200); x1(48)]: pieces h2A, h2hi73=[h2hi;ones], x1
    wf1k0 = W1[:, 0:HLO].T  # [128,200]
    wf1k1 = np.concatenate([W1[:, HLO:H], b1[:, None]], 1).T  # [73,200]
    # 49th row zero: px's ones row contributes nothing (b1 rides wf1k1)
    wf1k2 = np.concatenate([W1[:, H:H + DT], np.zeros((H, 1), np.float32)], 1).T  # [49,200]
    # fc2: pieces o3 (73 = o3hi(72)+ones), o3A (128)
    wf2k0 = W2[:, 0:HLO].T  # [128,48]
    wf2k1 = np.concatenate([W2[:, HLO:H], b2[:, None]], 1).T  # [73,48]

    ws = dict(wx1=wx1, wzs=wzs, wf1k0=wf1k0, wf1k1=wf1k1, wf1k2=wf1k2,
              wf2k0=wf2k0, wf2k1=wf2k1)
    out = {k: np.ascontiguousarray(v.astype(ml_dtypes.bfloat16)) for k, v in ws.items()}

    # DoubleRow fp8 weights [128, 2, 800] -> [128, 1600]
    # plane0 = h rows 0:128; plane1 = [h rows 128:200 (72); extras; zeros]
    e4 = ml_dtypes.float8_e4m3
    Wih1P, Whh1P = Wih1[P], Whh1[P]
    Wih2P, Whh2P = Wih2[P], Whh2[P]
    WactP, WstaP, bb1P, bb2P = Wact[P], Wsta[P], bb1[P], bb2[P]

    def drw(plane0_T, fills):
        w = np.zeros((128, 2, 800), np.float32)
        w[:, 0, :] = plane0_T
        for (r0, r1), blk in fills:
            w[r0:r1, 1, :] = blk
        return np.ascontiguousarray(w.reshape(128, 1600).astype(e4))

    out["wdr1"] = drw(Whh1P[:, 0:128].T,
                      [((0, 72), Whh1P[:, 128:200].T), ((72, 120), Wih1P.T),
                       ((120, 121), bb1P[None, :])])
    out["wdr2a"] = drw(Wih2P[:, 0:128].T,
                       [((0, 72), Wih2P[:, 128:200].T), ((72, 78), WactP.T),
                        ((78, 84), WstaP.T), ((84, 85), bb2P[None, :])])
    out["wdr2b"] = drw(Whh2P[:, 0:128].T,
                       [((0, 72), Whh2P[:, 128:200].T)])
    return out


def prep_core_inputs(tactiles, actions, cf, T, core, n_cores):
    B = tactiles.shape[1]
    Bc = B // n_cores
    sl = slice(core * Bc, (core + 1) * Bc)
    bf = ml_dtypes.bfloat16
    e4 = ml_dtypes.float8_e4m3
    # bf16 tactile for the x1t load at t = cf-1
    tact = np.ascontiguousarray(
        np.transpose(tactiles[:max(cf, 1), sl, :], (0, 2, 1)).astype(bf))
    at = np.transpose(actions[:, sl, :], (0, 2, 1))  # [T, 6, Bc]
    acts_f = np.concatenate(
        [at[1:T], np.broadcast_to(at[0:1], (T - 1, DA, Bc))], axis=1)
    acts = np.ascontiguousarray(acts_f.astype(bf))
    ones = np.ones((1, Bc), bf)
    # fp8 tactiles for all context steps t = 0..cf-1
    tact8 = np.ascontiguousarray(
        np.transpose(tactiles[:max(cf, 1), sl, :], (0, 2, 1)).astype(e4))
    # fp8 acts for all steps
    acts8 = np.ascontiguousarray(acts_f.astype(e4))
    ones8 = np.ones((1, Bc), e4)
    zero8 = np.zeros((128, Bc), e4)
    return dict(tact=tact, acts=acts, ones=ones, tact8=tact8, acts8=acts8,
                ones8=ones8, zero8=zero8)


def build_nc(Bc, cf, T=20, skew=3, fc_lag=2, gbufs=3, tbufs=5):
    assert cf >= 2, "kernel specialized for cf >= 2"
    nchunks = Bc // NC
    nsteps = T - 1
    nout = T - cf
    nc = bacc.Bacc(None)

    wshapes = dict(wx1=(49, 800), wzs=(13, 800), wf1k0=(128, 200),
                   wf1k1=(73, 200), wf1k2=(49, 200), wf2k0=(128, 48),
                   wf2k1=(73, 48))
    wext = {k: nc.declare_dram_parameter(k, list(s), BF, isOutput=False)
            for k, s in wshapes.items()}
    wshapes8 = {k: (128, 1600) for k in ("wdr1", "wdr2a", "wdr2b")}
    w8ext = {k: nc.declare_dram_parameter(k, list(s), E4, isOutput=False)
             for k, s in wshapes8.items()}
    tact_e = nc.declare_dram_parameter("tact", [max(cf, 1), DT, Bc], BF, isOutput=False)
    tact8_e = nc.declare_dram_parameter("tact8", [max(cf, 1), DT, Bc], E4, isOutput=False)
    acts8_e = nc.declare_dram_parameter("acts8", [nsteps, 2 * DA, Bc], E4, isOutput=False)
    acts_e = nc.declare_dram_parameter("acts", [nsteps, 2 * DA, Bc], BF, isOutput=False)
    ones8_e = nc.declare_dram_parameter("ones8", [1, Bc], E4, isOutput=False)
    zero8_e = nc.declare_dram_parameter("zero8", [128, Bc], E4, isOutput=False)
    ones_e = nc.declare_dram_parameter("ones", [1, Bc], BF, isOutput=False)
    out_e = nc.declare_dram_parameter("out", [nout, DT, Bc], BF, isOutput=True)

    with tile.TileContext(nc) as tc, contextlib.ExitStack() as ctx:
        wpool = ctx.enter_context(tc.tile_pool(name="w", bufs=1))
        state = ctx.enter_context(tc.tile_pool(name="state", bufs=1))
        gates = ctx.enter_context(tc.tile_pool(name="gates", bufs=gbufs))
        tmps = ctx.enter_context(tc.tile_pool(name="tmps", bufs=tbufs))
        ps3 = ctx.enter_context(tc.tile_pool(name="ps3", bufs=2, space="PSUM"))
        ps1 = ctx.enter_context(tc.tile_pool(name="ps1", bufs=2, space="PSUM"))

        wsb = {}
        for k, s in wshapes.items():
            wt = wpool.tile(list(s), BF, tag=k)
            nc.sync.dma_start(wt[:], wext[k][:])
            wsb[k] = wt
        for k, s in wshapes8.items():
            wt = wpool.tile(list(s), E4, tag=k, name=f"w8_{k}")
            nc.sync.dma_start(wt[:], w8ext[k][:])
            wsb[k] = wt

        # persistent state
        h2A = state.tile([HLO, Bc], BF, tag="h2A")
        h2hi = state.tile([73, Bc], BF, tag="h2hi")
        c1 = state.tile([HLO, 2 * Bc], BF, tag="c1")
        c2 = state.tile([HLO, 2 * Bc], BF, tag="c2")
        o3t = [state.tile([73, NC], BF, tag=f"o3_{p}", name=f"o3_{p}") for p in range(2)]
        # px: [x1(0:48); ones(48)] — z1 bf16 tail rhs; rows 0:48 double as x1t
        px = [state.tile([49, Bc], BF, tag=f"px_{p}", name=f"px_{p}") for p in range(2)]
        # hs: [acts(0:12); ones(12)] — z2 bf16 tail rhs
        hs = [state.tile([13, Bc], BF, tag=f"hs_{p}", name=f"hs_{p}") for p in range(2)]
        xz1 = state.tile([128, 3 * Bc], E4, tag="xz1")
        xh2 = state.tile([128, 2 * Bc], E4, tag="xh2")

        # init
        nc.vector.memset(c1[:], 0.0)
        nc.vector.memset(c2[:], 0.0)
        nc.vector.memset(h2A[:], 0.0)
        nc.vector.memset(h2hi[0:72, :], 0.0)
        nc.sync.dma_start(h2hi[72:73, :], ones_e[:])
        for p in range(2):
            nc.sync.dma_start(o3t[p][72:73, :], ones_e[:, 0:NC])
            nc.sync.dma_start(px[p][48:49, :], ones_e[:])
            nc.sync.dma_start(hs[p][12:13, :], ones_e[:])
        for _b in range(3):
            nc.sync.dma_start(xz1[:, _b * Bc:(_b + 1) * Bc], zero8_e[:])
        for _b in range(2):
            nc.sync.dma_start(xh2[:, _b * Bc:(_b + 1) * Bc], zero8_e[:])
        nc.sync.dma_start(xz1[120:121, Bc:2 * Bc], ones8_e[:])
        nc.sync.dma_start(xz1[84:85, 2 * Bc:3 * Bc], ones8_e[:])

        GOFF_LO = [0, 128, 256, 384]   # i,f,o,g lo col offsets in w tiles
        GOFF_HI = [512, 584, 656, 728]

        def z_mms_dr(rpieces, ch, tail=None):
            """DoubleRow fp8 matmuls: rpieces = [(wname, rhs3d)] with rhs3d
            [128, 2, Bc]-sliced per chunk. tail: optional [(wname, rhs2d)]
            bf16 pieces accumulated after the DR calls."""
            cs = slice(ch * NC, (ch + 1) * NC)
            s3l = ps3.tile([HLO, 3 * NC], F32, tag="s3")
            s3h = ps3.tile([HHI, 3 * NC], F32, tag="s3")
            s1l = ps1.tile([HLO, NC], F32, tag="s1")
            s1h = ps1.tile([HHI, NC], F32, tag="s1")
            tail = tail or []
            nk = len(rpieces) + len(tail)
            wr = {wn: wsb[wn][:].rearrange("p (two m) -> p two m", two=2)
                  for wn, _ in rpieces}

            def emit(slab, cols, mo, mw):
                ki = 0
                for wn, rap in rpieces:
                    nc.tensor.matmul(slab[:, cols], wr[wn][:, :, mo:mo + mw],
                                     rap[:, :, cs], start=(ki == 0),
                                     stop=(ki == nk - 1), perf_mode=DR)
                    ki += 1
                for wn, rhs in tail:
                    nc.tensor.matmul(slab[:, cols], wsb[wn][:, mo:mo + mw],
                                     rhs[:, cs], start=(ki == 0),
                                     stop=(ki == nk - 1))
                    ki += 1

            for gi in range(3):
                for slab, offs, mw in ((s3l, GOFF_LO, HLO), (s3h, GOFF_HI, HHI)):
                    emit(slab, slice(gi * NC, (gi + 1) * NC), offs[gi], mw)
            for slab, offs, mw in ((s1l, GOFF_LO, HLO), (s1h, GOFF_HI, HHI)):
                emit(slab, slice(0, NC), offs[3], mw)
            return s3l, s3h, s1l, s1h

        xz1r = xz1[:].rearrange("p (blk b) -> p blk b", blk=3)
        xh2r = xh2[:].rearrange("p (blk b) -> p blk b", blk=2)
        xz1_01 = xz1r[:, 0:2, :]    # (b0, b1): z1 rhs
        xz1_02 = xz1r[:, 0:3:2, :]  # (b0, b2): z2 fold1 rhs
        xh2_01 = xh2r               # (b0, b1): z2 fold2 rhs

        def cpair(c_t, ch):
            r = c_t[:].rearrange("p (two b) -> p two b", two=2)
            return r[:, :, ch * NC:(ch + 1) * NC]

        def lstm_elem(slabs, ch, c_t, hdst_lo, hdst_hi, copy_dst):
            """Gate nonlinearities + cell update, unified gates tile.
            g layout: [i_lo f_lo o_lo | i_hi f_hi o_hi | g_lo g_hi] x NC."""
            s3l, s3h, s1l, s1h = slabs
            g = gates.tile([HLO, 8 * NC], BF, tag="g")
            nc.scalar.activation(g[:, 0:3 * NC], s3l[:], F.Sigmoid)
            nc.scalar.activation(g[0:HHI, 3 * NC:6 * NC], s3h[:], F.Sigmoid)
            nc.scalar.activation(g[:, 6 * NC:7 * NC], s1l[:], F.Tanh)
            nc.scalar.activation(g[0:HHI, 7 * NC:8 * NC], s1h[:], F.Tanh)
            t_ig = tmps.tile([HLO, 2 * NC], BF, tag="t_ig")
            t_fc = tmps.tile([HLO, 2 * NC], BF, tag="t_fc")
            t_tc = tmps.tile([HLO, 2 * NC], BF, tag="t_tc")
            cap = cpair(c_t, ch)
            c_lo = c_t[:, ch * NC:(ch + 1) * NC]
            c_hi = c_t[0:HHI, Bc + ch * NC:Bc + (ch + 1) * NC]
            # i*g
            nc.gpsimd.tensor_tensor(t_ig[:, 0:NC], g[:, 0:NC], g[:, 6 * NC:7 * NC], A.mult)
            nc.gpsimd.tensor_tensor(t_ig[0:HHI, NC:2 * NC], g[0:HHI, 3 * NC:4 * NC],
                                    g[0:HHI, 7 * NC:8 * NC], A.mult)
            # f*c
            nc.vector.tensor_tensor(t_fc[:, 0:NC], g[:, NC:2 * NC], c_lo, A.mult)
            nc.vector.tensor_tensor(t_fc[0:HHI, NC:2 * NC], g[0:HHI, 4 * NC:5 * NC], c_hi, A.mult)
            # c = ig + fc
            nc.vector.tensor_tensor(c_lo, t_ig[:, 0:NC], t_fc[:, 0:NC], A.add)
            nc.vector.tensor_tensor(c_hi, t_ig[0:HHI, NC:2 * NC], t_fc[0:HHI, NC:2 * NC], A.add)
            # tanh(c) merged (hi pad partitions stay zero from init)
            nc.scalar.activation(t_tc[:], cap, F.Tanh)
            nc.vector.tensor_tensor(hdst_lo, g[:, 2 * NC:3 * NC], t_tc[:, 0:NC], A.mult)
            nc.vector.tensor_tensor(hdst_hi, g[0:HHI, 5 * NC:6 * NC], t_tc[0:HHI, NC:2 * NC], A.mult)
            if copy_dst is not None:
                nc.vector.tensor_copy(copy_dst, hdst_hi)

        def emit_lstm1(t, ch):
            cs = slice(ch * NC, (ch + 1) * NC)
            # prediction steps: x1 (fed-back prediction) + bias enter in
            # bf16; the fp8 x/ones rows of xz1 b1 are zeroed at t=cf so
            # wdr1's Wih1/bias planes contribute nothing.
            tail = [("wx1", px[t % 2])] if t >= cf else None
            slabs = z_mms_dr([("wdr1", xz1_01)], ch, tail=tail)
            lstm_elem(slabs, ch, c1, xz1[:, cs],
                      xz1[0:72, Bc + ch * NC:Bc + (ch + 1) * NC],
                      xz1[0:72, 2 * Bc + ch * NC:2 * Bc + (ch + 1) * NC])

        def emit_lstm2(t, ch):
            cs = slice(ch * NC, (ch + 1) * NC)
            # t >= cf-1: acts/state/bias in bf16 (b2's fp8 copies zeroed)
            tail = [("wzs", hs[t % 2])] if t >= cf - 1 else None
            slabs = z_mms_dr([("wdr2a", xz1_02), ("wdr2b", xh2_01)], ch, tail=tail)
            if t < cf - 1:
                # pure-fp8 context: h2 straight into fp8 planes
                lstm_elem(slabs, ch, c2, xh2[:, cs],
                          xh2[0:72, Bc + ch * NC:Bc + (ch + 1) * NC], None)
            else:
                # fc needs bf16 h2; also refresh fp8 planes for next z2
                lstm_elem(slabs, ch, c2, h2A[:, cs], h2hi[0:72, cs], None)
                if t + 1 < nsteps:
                    nc.gpsimd.tensor_copy(xh2[:, cs], h2A[:, cs])
                    nc.gpsimd.tensor_copy(xh2[0:72, Bc + ch * NC:Bc + (ch + 1) * NC],
                                          h2hi[0:72, cs])

        def emit_fc(t, ch):
            cs = slice(ch * NC, (ch + 1) * NC)
            xcur, xnxt = px[t % 2], px[(t + 1) % 2]
            f1l = ps1.tile([HLO, NC], F32, tag="s1")
            f1h = ps1.tile([HHI, NC], F32, tag="s1")
            pieces = [("wf1k0", h2A), ("wf1k1", h2hi), ("wf1k2", xcur)]
            for ki, (wn, rhs) in enumerate(pieces):
                nc.tensor.matmul(f1l[:], wsb[wn][:, 0:HLO], rhs[:, cs],
                                 start=(ki == 0), stop=(ki == 2))
            for ki, (wn, rhs) in enumerate(pieces):
                nc.tensor.matmul(f1h[:], wsb[wn][:, HLO:H], rhs[:, cs],
                                 start=(ki == 0), stop=(ki == 2))
            o3 = o3t[ch % 2]
            o3A = gates.tile([HLO, NC], BF, tag="o3A")
            nc.scalar.activation(o3A[:], f1l[:], F.Tanh)
            nc.scalar.activation(o3[0:72, :], f1h[:], F.Tanh)
            f2 = ps1.tile([DT, NC], F32, tag="s1")
            p2 = [("wf2k0", o3A[:]), ("wf2k1", o3[:])]
            for ki, (wn, rhs) in enumerate(p2):
                nc.tensor.matmul(f2[:], wsb[wn][:, 0:DT], rhs,
                                 start=(ki == 0), stop=(ki == 1))
            nc.scalar.activation(xnxt[0:48, cs], f2[:], F.Tanh)


        for t in range(nsteps):
            if t <= cf - 1:
                nc.sync.dma_start(xz1[72:120, Bc:2 * Bc], tact8_e[t])
                if t == cf - 1:
                    nc.sync.dma_start(px[t % 2][0:48, :], tact_e[t])
            elif t == cf:
                # retire the fp8 x+ones rows: pred x1+bias ride the bf16 tail
                nc.sync.dma_start(xz1[72:121, Bc:2 * Bc], zero8_e[0:49, :])
            if t < cf - 1:
                nc.sync.dma_start(xz1[72:84, 2 * Bc:3 * Bc], acts8_e[t])
            else:
                if t == cf - 1:
                    # retire fp8 acts/state/bias rows of b2
                    nc.sync.dma_start(xz1[72:85, 2 * Bc:3 * Bc], zero8_e[0:13, :])
                nc.sync.dma_start(hs[t % 2][0:12, :], acts_e[t])
            fc_step = t >= cf - 1
            for ch in range(nchunks + skew + (fc_lag if fc_step else 0)):
                if ch < nchunks:
                    emit_lstm1(t, ch)
                if skew <= ch < nchunks + skew:
                    emit_lstm2(t, ch - skew)
                if fc_step and ch >= skew + fc_lag:
                    emit_fc(t, ch - skew - fc_lag)
            if fc_step:
                nc.sync.dma_start(out_e[t - (cf - 1)], px[(t + 1) % 2][0:48, :])

    nc.finalize()
    return nc


def reorder_outputs(res_out, B, n_cores, nout):
    full = np.concatenate([np.transpose(np.float32(r), (0, 2, 1)) for r in res_out],
                          axis=1)
    return np.ascontiguousarray(full)


_BUILD_CACHE = {}


def kernel(tactiles, actions, Wih1, Whh1, bih1, bhh1, Wih2, Whh2, bih2, bhh2,
           W1, b1, W2, b2, context_frames):
    from concourse.bass_utils import run_bass_kernel_spmd

    tactiles = np.asarray(tactiles)
    actions = np.asarray(actions)
    cf = int(np.asarray(context_frames))
    T, B, _ = tactiles.shape
    n_cores = 8
    Bc = B // n_cores

    key = (Bc, cf, T)
    if key not in _BUILD_CACHE:
        _BUILD_CACHE[key] = build_nc(Bc=Bc, cf=cf, T=T)
    nc = _BUILD_CACHE[key]

    inp = dict(Wih1=Wih1, Whh1=Whh1, bih1=bih1, bhh1=bhh1, Wih2=Wih2, Whh2=Whh2,
               bih2=bih2, bhh2=bhh2, W1=W1, b1=b1, W2=W2, b2=b2)
    ws = prep_weights(inp)
    in_maps = []
    for core in range(n_cores):
        m = dict(ws)
        m.update(prep_core_inputs(tactiles, actions, cf, T, core, n_cores))
        in_maps.append(m)

    res = run_bass_kernel_spmd(nc, in_maps, core_ids=list(range(n_cores)))
    out = reorder_outputs([r["out"] for r in res.results], B, n_cores, T - cf)
    return out.astype(np.float32)


# revision 27
# speedup vs baseline: 1.3619x; 1.3619x over previous
"""Stage A rewrite: bf16 cell, K-packed z1/fc2, unified gate tiles, merged
elementwise ops, ping-pong step buffers.

Layouts (per core, Bc=4096, chunks of NC=512, feature-major [feat, batch]):
- xh1 (x2 ping-pong) [121, Bc] bf16: rows 0:48 x1, 48 ones, 49:121 h1_hi.
- h1A [128, Bc] bf16 (h1 rows 0:128).
- hs1 (x2 ping-pong) [85, Bc] bf16: rows 0:72 h1_hi copy, 72:84 acts, 84 ones.
- h2A [128, Bc], h2hi [73, Bc] (72 rows + ones) bf16.
- c1, c2 [128, 2*Bc] bf16: cols 0:Bc = H rows 0:128, Bc:2Bc = H rows 128:200
  (partitions 0:72).
- gates g1, g2 per chunk [128, 4096] bf16: per gate 1024 cols (lo 512 | hi 512),
  gate order i,f,o,g.
- PSUM slabs unchanged: s3l [128,1536], s3h [72,1536] (i,f,o), s1l/s1h (g).
"""
import sys as _sys
for _p in ("/opt/trn_rl_repo", "/root/.axon_site/_ro/trn_rl_repo"):
    if _p not in _sys.path:
        _sys.path.append(_p)

import numpy as np
import ml_dtypes
import contextlib

import concourse.bass as bass
import concourse.tile as tile
from concourse import bacc, mybir

F = mybir.ActivationFunctionType
A = mybir.AluOpType
BF = mybir.dt.bfloat16
F32 = mybir.dt.float32
E4 = mybir.dt.float8e4
DR = mybir.MatmulPerfMode.DoubleRow

H, DT, DA = 200, 48, 6
HLO, HHI = 128, 72
NC = 512


def perm_indices():
    # gate order i,f,o,g; each gate split lo(128)+hi(72)
    gr = {"i": 0, "f": 200, "o": 600, "g": 400}
    order = ["i", "f", "o", "g"]
    lo = np.concatenate([np.arange(gr[g], gr[g] + HLO) for g in order])
    hi = np.concatenate([np.arange(gr[g] + HLO, gr[g] + H) for g in order])
    return np.concatenate([lo, hi])


def prep_weights(inp):
    P = perm_indices()
    f32 = np.float32
    Wih1, Whh1 = f32(inp["Wih1"]), f32(inp["Whh1"])
    Wih2, Whh2 = f32(inp["Wih2"]), f32(inp["Whh2"])
    W1, W2 = f32(inp["W1"]), f32(inp["W2"])
    b1, b2 = f32(inp["b1"]), f32(inp["b2"])
    bb1 = f32(inp["bih1"]) + f32(inp["bhh1"])
    bb2 = f32(inp["bih2"]) + f32(inp["bhh2"])

    # z1 pieces: xh1=[h1hi(72); x1(48); ones] (121), h1A (128)
    wz1k0 = np.concatenate([Whh1[:, HLO:H], Wih1, bb1[:, None]], 1)[P].T  # [121,800]
    wz1k1 = Whh1[:, 0:HLO][P].T  # [128,800]

    # z2 pieces: h1A (128), hs1=[h1hi(72); act(6); state(6); ones] (85),
    # h2hi[0:72] (72), h2A (128)
    Wt = Wih2[:, H:H + DT]
    Wact = Wt[:, 0:6] + Wt[:, 12:18] + Wt[:, 24:30] + Wt[:, 36:42]
    Wsta = Wt[:, 6:12] + Wt[:, 18:24] + Wt[:, 30:36] + Wt[:, 42:48]
    wz2k0 = Wih2[:, 0:HLO][P].T
    wz2k1 = np.concatenate([Wih2[:, HLO:H], Wact, Wsta, bb2[:, None]], 1)[P].T  # [85,800]
    wz2k2 = Whh2[:, HLO:H][P].T  # [72,800]
    wz2k3 = Whh2[:, 0:HLO][P].T  # [128,800]

    # fc1: lp=[h2(200); x1(48)]: pieces h2A, h2hi73=[h2hi;ones], xh1[0:48]
    wf1k0 = W1[:, 0:HLO].T  # [128,200]
    wf1k1 = np.concatenate([W1[:, HLO:H], b1[:, None]], 1).T  # [73,200]
    wf1k2 = W1[:, H:H + DT].T  # [48,200]
    # fc2: pieces o3 (73 = o3hi(72)+ones), o3A (128)
    wf2k0 = W2[:, 0:HLO].T  # [128,48]
    wf2k1 = np.concatenate([W2[:, HLO:H], b2[:, None]], 1).T  # [73,48]

    ws = dict(wz1k0=wz1k0, wz1k1=wz1k1, wz2k0=wz2k0, wz2k1=wz2k1,
              wz2k2=wz2k2, wz2k3=wz2k3, wf1k0=wf1k0, wf1k1=wf1k1, wf1k2=wf1k2,
              wf2k0=wf2k0, wf2k1=wf2k1)
    out = {k: np.ascontiguousarray(v.astype(ml_dtypes.bfloat16)) for k, v in ws.items()}

    # DoubleRow fp8 weights [128, 2, 800] -> [128, 1600]
    # plane0 = h rows 0:128; plane1 = [h rows 128:200 (72); extras; zeros]
    e4 = ml_dtypes.float8_e4m3
    Wih1P, Whh1P = Wih1[P], Whh1[P]
    Wih2P, Whh2P = Wih2[P], Whh2[P]
    WactP, WstaP, bb1P, bb2P = Wact[P], Wsta[P], bb1[P], bb2[P]

    def drw(plane0_T, fills):
        w = np.zeros((128, 2, 800), np.float32)
        w[:, 0, :] = plane0_T
        for (r0, r1), blk in fills:
            w[r0:r1, 1, :] = blk
        return np.ascontiguousarray(w.reshape(128, 1600).astype(e4))

    out["wdr1"] = drw(Whh1P[:, 0:128].T,
                      [((0, 72), Whh1P[:, 128:200].T), ((72, 120), Wih1P.T),
                       ((120, 121), bb1P[None, :])])
    out["wdr2a"] = drw(Wih2P[:, 0:128].T,
                       [((0, 72), Wih2P[:, 128:200].T), ((72, 78), WactP.T),
                        ((78, 84), WstaP.T), ((84, 85), bb2P[None, :])])
    out["wdr2b"] = drw(Whh2P[:, 0:128].T,
                       [((0, 72), Whh2P[:, 128:200].T)])
    return out


def prep_core_inputs(tactiles, actions, cf, T, core, n_cores):
    B = tactiles.shape[1]
    Bc = B // n_cores
    sl = slice(core * Bc, (core + 1) * Bc)
    bf = ml_dtypes.bfloat16
    tact = np.ascontiguousarray(
        np.transpose(tactiles[:cf, sl, :], (0, 2, 1)).astype(bf))
    at = np.transpose(actions[:, sl, :], (0, 2, 1))  # [T, 6, Bc]
    acts = np.concatenate(
        [at[1:T], np.broadcast_to(at[0:1], (T - 1, DA, Bc))], axis=1)
    acts = np.ascontiguousarray(acts.astype(bf))
    ones = np.ones((1, Bc), bf)
    e4 = ml_dtypes.float8_e4m3
    nfp8 = max(cf - 1, 1)
    tact8 = np.ascontiguousarray(
        np.transpose(tactiles[:nfp8, sl, :], (0, 2, 1)).astype(e4))
    acts8 = np.ascontiguousarray(acts[:nfp8].astype(e4))
    ones8 = np.ones((1, Bc), e4)
    zero8 = np.zeros((128, Bc), e4)
    return dict(tact=tact, acts=acts, ones=ones, tact8=tact8, acts8=acts8,
                ones8=ones8, zero8=zero8)


def build_nc(Bc, cf, T=20, skew=3, fc_lag=2, gbufs=3, tbufs=5):
    nchunks = Bc // NC
    nsteps = T - 1
    nout = T - cf
    nc = bacc.Bacc(None)

    wshapes = dict(wz1k0=(121, 800), wz1k1=(128, 800),
                   wz2k0=(128, 800), wz2k1=(85, 800), wz2k2=(72, 800),
                   wz2k3=(128, 800),
                   wf1k0=(128, 200), wf1k1=(73, 200), wf1k2=(48, 200),
                   wf2k0=(128, 48), wf2k1=(73, 48))
    wext = {k: nc.declare_dram_parameter(k, list(s), BF, isOutput=False)
            for k, s in wshapes.items()}
    wshapes8 = {}
    tact_e = nc.declare_dram_parameter("tact", [max(cf, 1), DT, Bc], BF, isOutput=False)
    nfp8 = max(cf - 1, 1)
    for _k in ("wdr1", "wdr2a", "wdr2b"):
        wshapes8[_k] = (128, 1600)
    w8ext = {k: nc.declare_dram_parameter(k, list(s), E4, isOutput=False)
             for k, s in wshapes8.items()}
    tact8_e = nc.declare_dram_parameter("tact8", [nfp8, DT, Bc], E4, isOutput=False)
    acts8_e = nc.declare_dram_parameter("acts8", [nfp8, 2 * DA, Bc], E4, isOutput=False)
    ones8_e = nc.declare_dram_parameter("ones8", [1, Bc], E4, isOutput=False)
    zero8_e = nc.declare_dram_parameter("zero8", [128, Bc], E4, isOutput=False)
    ones_e = nc.declare_dram_parameter("ones", [1, Bc], BF, isOutput=False)
    acts_e = nc.declare_dram_parameter("acts", [nsteps, 2 * DA, Bc], BF, isOutput=False)
    out_e = nc.declare_dram_parameter("out", [nout, DT, Bc], BF, isOutput=True)

    with tile.TileContext(nc) as tc, contextlib.ExitStack() as ctx:
        wpool = ctx.enter_context(tc.tile_pool(name="w", bufs=1))
        state = ctx.enter_context(tc.tile_pool(name="state", bufs=1))
        gates = ctx.enter_context(tc.tile_pool(name="gates", bufs=gbufs))
        tmps = ctx.enter_context(tc.tile_pool(name="tmps", bufs=tbufs))
        ps3 = ctx.enter_context(tc.tile_pool(name="ps3", bufs=2, space="PSUM"))
        ps1 = ctx.enter_context(tc.tile_pool(name="ps1", bufs=2, space="PSUM"))

        wsb = {}
        for k, s in wshapes.items():
            wt = wpool.tile(list(s), BF, tag=k)
            nc.sync.dma_start(wt[:], wext[k][:])
            wsb[k] = wt
        for k, s in wshapes8.items():
            wt = wpool.tile(list(s), E4, tag=k, name=f"w8_{k}")
            nc.sync.dma_start(wt[:], w8ext[k][:])
            wsb[k] = wt

        # persistent state
        xh1 = [state.tile([128, Bc], BF, tag=f"xh1_{p}", name=f"xh1_{p}") for p in range(2)]
        hs1 = [state.tile([85, Bc], BF, tag=f"hs1_{p}", name=f"hs1_{p}") for p in range(2)]
        h1A = state.tile([HLO, Bc], BF, tag="h1A")
        h2A = state.tile([HLO, Bc], BF, tag="h2A")
        h2hi = state.tile([73, Bc], BF, tag="h2hi")
        c1 = state.tile([HLO, 2 * Bc], BF, tag="c1")
        c2 = state.tile([HLO, 2 * Bc], BF, tag="c2")
        o3t = [state.tile([73, NC], BF, tag=f"o3_{p}", name=f"o3_{p}") for p in range(2)]
        x1t = [state.tile([48, Bc], BF, tag=f"x1t_{p}", name=f"x1t_{p}") for p in range(2)]
        xz1 = state.tile([128, 3 * Bc], E4, tag="xz1")
        xh2 = state.tile([128, 2 * Bc], E4, tag="xh2")

        # init
        nc.vector.memset(c1[:], 0.0)
        nc.vector.memset(c2[:], 0.0)
        nc.vector.memset(h1A[:], 0.0)
        nc.vector.memset(h2A[:], 0.0)
        nc.vector.memset(h2hi[0:72, :], 0.0)
        for p in range(2):
            nc.vector.memset(xh1[p][0:72, :], 0.0)
            nc.vector.memset(hs1[p][0:72, :], 0.0)
            nc.sync.dma_start(xh1[p][120:121, :], ones_e[:])
            nc.sync.dma_start(hs1[p][84:85, :], ones_e[:])
            nc.sync.dma_start(o3t[p][72:73, :], ones_e[:, 0:NC])
        nc.sync.dma_start(h2hi[72:73, :], ones_e[:])
        for _b in range(3):
            nc.sync.dma_start(xz1[:, _b * Bc:(_b + 1) * Bc], zero8_e[:])
        for _b in range(2):
            nc.sync.dma_start(xh2[:, _b * Bc:(_b + 1) * Bc], zero8_e[:])
        nc.sync.dma_start(xz1[120:121, Bc:2 * Bc], ones8_e[:])
        nc.sync.dma_start(xz1[84:85, 2 * Bc:3 * Bc], ones8_e[:])
        if cf == 0:
            for p in range(2):
                nc.vector.memset(xh1[p][72:120, :], 0.0)

        GOFF_LO = [0, 128, 256, 384]   # i,f,o,g lo col offsets in w tiles
        GOFF_HI = [512, 584, 656, 728]

        def z_mms(kpieces, ch):
            """Gate matmuls for one z, chunk ch. Returns psum slabs."""
            cs = slice(ch * NC, (ch + 1) * NC)
            s3l = ps3.tile([HLO, 3 * NC], F32, tag="s3")
            s3h = ps3.tile([HHI, 3 * NC], F32, tag="s3")
            s1l = ps1.tile([HLO, NC], F32, tag="s1")
            s1h = ps1.tile([HHI, NC], F32, tag="s1")
            nk = len(kpieces)
            for gi in range(3):  # i, f, o
                for slab, offs, mw in ((s3l, GOFF_LO, HLO), (s3h, GOFF_HI, HHI)):
                    mo = offs[gi]
                    for ki, (wn, rhs) in enumerate(kpieces):
                        nc.tensor.matmul(
                            slab[:, gi * NC:(gi + 1) * NC],
                            wsb[wn][:, mo:mo + mw], rhs[:, cs],
                            start=(ki == 0), stop=(ki == nk - 1))
            for slab, offs, mw in ((s1l, GOFF_LO, HLO), (s1h, GOFF_HI, HHI)):
                mo = offs[3]
                for ki, (wn, rhs) in enumerate(kpieces):
                    nc.tensor.matmul(slab[:], wsb[wn][:, mo:mo + mw], rhs[:, cs],
                                     start=(ki == 0), stop=(ki == nk - 1))
            return s3l, s3h, s1l, s1h

        def z_mms_dr(rpieces, ch):
            """DoubleRow fp8 matmuls: rpieces = [(wname, rhs3d)] with rhs3d
            [128, 2, Bc]-sliced per chunk. Same slab structure as z_mms."""
            cs = slice(ch * NC, (ch + 1) * NC)
            s3l = ps3.tile([HLO, 3 * NC], F32, tag="s3")
            s3h = ps3.tile([HHI, 3 * NC], F32, tag="s3")
            s1l = ps1.tile([HLO, NC], F32, tag="s1")
            s1h = ps1.tile([HHI, NC], F32, tag="s1")
            nk = len(rpieces)
            wr = {wn: wsb[wn][:].rearrange("p (two m) -> p two m", two=2)
                  for wn, _ in rpieces}
            for gi in range(3):
                for slab, offs, mw in ((s3l, GOFF_LO, HLO), (s3h, GOFF_HI, HHI)):
                    mo = offs[gi]
                    for ki, (wn, rap) in enumerate(rpieces):
                        nc.tensor.matmul(
                            slab[:, gi * NC:(gi + 1) * NC],
                            wr[wn][:, :, mo:mo + mw], rap[:, :, cs],
                            start=(ki == 0), stop=(ki == nk - 1), perf_mode=DR)
            for slab, offs, mw in ((s1l, GOFF_LO, HLO), (s1h, GOFF_HI, HHI)):
                mo = offs[3]
                for ki, (wn, rap) in enumerate(rpieces):
                    nc.tensor.matmul(slab[:], wr[wn][:, :, mo:mo + mw], rap[:, :, cs],
                                     start=(ki == 0), stop=(ki == nk - 1), perf_mode=DR)
            return s3l, s3h, s1l, s1h

        xz1r = xz1[:].rearrange("p (blk b) -> p blk b", blk=3)
        xh2r = xh2[:].rearrange("p (blk b) -> p blk b", blk=2)
        xz1_01 = xz1r[:, 0:2, :]    # (b0, b1): z1 rhs
        xz1_02 = xz1r[:, 0:3:2, :]  # (b0, b2): z2 fold1 rhs
        xh2_01 = xh2r               # (b0, b1): z2 fold2 rhs

        def cpair(c_t, ch):
            """3D AP covering (lo cols, hi cols) of chunk ch in a c tile."""
            r = c_t[:].rearrange("p (two b) -> p two b", two=2)
            return r[:, :, ch * NC:(ch + 1) * NC]

        def lstm_elem(slabs, ch, c_t, hdst_lo, hdst_hi, copy_dst):
            """Gate nonlinearities + cell update, unified gates tile.
            g layout: [i_lo f_lo o_lo | i_hi f_hi o_hi | g_lo g_hi] x NC."""
            s3l, s3h, s1l, s1h = slabs
            g = gates.tile([HLO, 8 * NC], BF, tag="g")
            gr = g[:].rearrange("p (blk b) -> p blk b", b=NC)
            nc.scalar.activation(g[:, 0:3 * NC], s3l[:], F.Sigmoid)
            nc.scalar.activation(g[0:HHI, 3 * NC:6 * NC], s3h[:], F.Sigmoid)
            nc.scalar.activation(g[:, 6 * NC:7 * NC], s1l[:], F.Tanh)
            nc.scalar.activation(g[0:HHI, 7 * NC:8 * NC], s1h[:], F.Tanh)
            t_ig = tmps.tile([HLO, 2 * NC], BF, tag="t_ig")
            t_fc = tmps.tile([HLO, 2 * NC], BF, tag="t_fc")
            t_tc = tmps.tile([HLO, 2 * NC], BF, tag="t_tc")
            cap = cpair(c_t, ch)
            c_lo = c_t[:, ch * NC:(ch + 1) * NC]
            c_hi = c_t[0:HHI, Bc + ch * NC:Bc + (ch + 1) * NC]
            # i*g
            nc.gpsimd.tensor_tensor(t_ig[:, 0:NC], g[:, 0:NC], g[:, 6 * NC:7 * NC], A.mult)
            nc.gpsimd.tensor_tensor(t_ig[0:HHI, NC:2 * NC], g[0:HHI, 3 * NC:4 * NC],
                                    g[0:HHI, 7 * NC:8 * NC], A.mult)
            # f*c
            nc.vector.tensor_tensor(t_fc[:, 0:NC], g[:, NC:2 * NC], c_lo, A.mult)
            nc.vector.tensor_tensor(t_fc[0:HHI, NC:2 * NC], g[0:HHI, 4 * NC:5 * NC], c_hi, A.mult)
            # c = ig + fc
            nc.vector.tensor_tensor(c_lo, t_ig[:, 0:NC], t_fc[:, 0:NC], A.add)
            nc.vector.tensor_tensor(c_hi, t_ig[0:HHI, NC:2 * NC], t_fc[0:HHI, NC:2 * NC], A.add)
            # tanh(c) merged (hi pad partitions stay zero from init)
            nc.scalar.activation(t_tc[:], cap, F.Tanh)
            nc.vector.tensor_tensor(hdst_lo, g[:, 2 * NC:3 * NC], t_tc[:, 0:NC], A.mult)
            nc.vector.tensor_tensor(hdst_hi, g[0:HHI, 5 * NC:6 * NC], t_tc[0:HHI, NC:2 * NC], A.mult)
            if copy_dst is not None:
                nc.vector.tensor_copy(copy_dst, hdst_hi)

        def emit_lstm1(t, ch, fp8, trans):
            cs = slice(ch * NC, (ch + 1) * NC)
            xnxt = xh1[(t + 1) % 2]
            if fp8:
                slabs = z_mms_dr([("wdr1", xz1_01)], ch)
                if trans:
                    # last fp8 step: h1 -> bf16 state (for t+1) + fp8 copies
                    # (for this step's fp8 z2 fold1 = {b0, b2})
                    lstm_elem(slabs, ch, c1, h1A[:, cs], xnxt[0:72, cs], None)
                    nc.vector.tensor_copy(xz1[:, cs], h1A[:, cs])
                    nc.vector.tensor_copy(xz1[0:72, 2 * Bc + ch * NC:2 * Bc + (ch + 1) * NC],
                                          xnxt[0:72, cs])
                else:
                    lstm_elem(slabs, ch, c1, xz1[:, cs], xz1[0:72, Bc + ch * NC:Bc + (ch + 1) * NC],
                              xz1[0:72, 2 * Bc + ch * NC:2 * Bc + (ch + 1) * NC])
            else:
                xcur = xh1[t % 2]
                slabs = z_mms([("wz1k0", xcur[0:121, :]), ("wz1k1", h1A)], ch)
                lstm_elem(slabs, ch, c1, h1A[:, cs], xnxt[0:72, cs],
                          hs1[t % 2][0:72, cs])

        def emit_lstm2(t, ch, fp8, trans):
            cs = slice(ch * NC, (ch + 1) * NC)
            if fp8:
                slabs = z_mms_dr([("wdr2a", xz1_02), ("wdr2b", xh2_01)], ch)
                if trans:
                    lstm_elem(slabs, ch, c2, h2A[:, cs], h2hi[0:72, cs], None)
                else:
                    lstm_elem(slabs, ch, c2, xh2[:, cs], xh2[0:72, Bc + ch * NC:Bc + (ch + 1) * NC],
                              None)
            else:
                slabs = z_mms([("wz2k0", h1A), ("wz2k1", hs1[t % 2]),
                               ("wz2k2", h2hi[0:72, :]), ("wz2k3", h2A)], ch)
                lstm_elem(slabs, ch, c2, h2A[:, cs], h2hi[0:72, cs], None)

        def emit_fc(t, ch):
            cs = slice(ch * NC, (ch + 1) * NC)
            xcur, xnxt = x1t[t % 2], x1t[(t + 1) % 2]
            f1l = ps1.tile([HLO, NC], F32, tag="s1")
            f1h = ps1.tile([HHI, NC], F32, tag="s1")
            pieces = [("wf1k0", h2A), ("wf1k1", h2hi), ("wf1k2", xcur)]
            for ki, (wn, rhs) in enumerate(pieces):
                nc.tensor.matmul(f1l[:], wsb[wn][:, 0:HLO], rhs[:, cs],
                                 start=(ki == 0), stop=(ki == 2))
            for ki, (wn, rhs) in enumerate(pieces):
                nc.tensor.matmul(f1h[:], wsb[wn][:, HLO:H], rhs[:, cs],
                                 start=(ki == 0), stop=(ki == 2))
            o3 = o3t[ch % 2]
            o3A = gates.tile([HLO, NC], BF, tag="o3A")
            nc.scalar.activation(o3A[:], f1l[:], F.Tanh)
            nc.scalar.activation(o3[0:72, :], f1h[:], F.Tanh)
            f2 = ps1.tile([DT, NC], F32, tag="s1")
            p2 = [("wf2k0", o3A[:]), ("wf2k1", o3[:])]
            for ki, (wn, rhs) in enumerate(p2):
                nc.tensor.matmul(f2[:], wsb[wn][:, 0:DT], rhs,
                                 start=(ki == 0), stop=(ki == 1))
            nc.scalar.activation(xnxt[:, cs], f2[:], F.Tanh)
            if t + 1 < nsteps:
                nc.sync.dma_start(xh1[(t + 1) % 2][72:120, cs], xnxt[:, cs])

        fp8_on = cf >= 3
        for t in range(nsteps):
            fp8 = fp8_on and t < cf - 1
            trans = fp8 and t == cf - 2
            if fp8:
                nc.sync.dma_start(xz1[72:120, Bc:2 * Bc], tact8_e[t])
                nc.sync.dma_start(xz1[72:84, 2 * Bc:3 * Bc], acts8_e[t])
            else:
                if t < cf:
                    nc.sync.dma_start(xh1[t % 2][72:120, :], tact_e[t])
                    if t == cf - 1:
                        nc.sync.dma_start(x1t[t % 2][:], tact_e[t])
                nc.sync.dma_start(hs1[t % 2][72:84, :], acts_e[t])
            fc_step = t >= cf - 1
            for ch in range(nchunks + skew + (fc_lag if fc_step else 0)):
                if ch < nchunks:
                    emit_lstm1(t, ch, fp8, trans)
                if skew <= ch < nchunks + skew:
                    emit_lstm2(t, ch - skew, fp8, trans)
                if fc_step and ch >= skew + fc_lag:
                    emit_fc(t, ch - skew - fc_lag)
            if fc_step:
                nc.sync.dma_start(out_e[t - (cf - 1)], x1t[(t + 1) % 2][:])

    nc.finalize()
    return nc


def reorder_outputs(res_out, B, n_cores, nout):
    full = np.concatenate([np.transpose(np.float32(r), (0, 2, 1)) for r in res_out],
                          axis=1)
    return np.ascontiguousarray(full)


_BUILD_CACHE = {}


def kernel(tactiles, actions, Wih1, Whh1, bih1, bhh1, Wih2, Whh2, bih2, bhh2,
           W1, b1, W2, b2, context_frames):
    from concourse.bass_utils import run_bass_kernel_spmd

    tactiles = np.asarray(tactiles)
    actions = np.asarray(actions)
    cf = int(np.asarray(context_frames))
    T, B, _ = tactiles.shape
    n_cores = 8
    Bc = B // n_cores

    key = (Bc, cf, T)
    if key not in _BUILD_CACHE:
        _BUILD_CACHE[key] = build_nc(Bc=Bc, cf=cf, T=T)
    nc = _BUILD_CACHE[key]

    inp = dict(Wih1=Wih1, Whh1=Whh1, bih1=bih1, bhh1=bhh1, Wih2=Wih2, Whh2=Whh2,
               bih2=bih2, bhh2=bhh2, W1=W1, b1=b1, W2=W2, b2=b2)
    ws = prep_weights(inp)
    in_maps = []
    for core in range(n_cores):
        m = dict(ws)
        m.update(prep_core_inputs(tactiles, actions, cf, T, core, n_cores))
        in_maps.append(m)

    res = run_bass_kernel_spmd(nc, in_maps, core_ids=list(range(n_cores)))
    out = reorder_outputs([r["out"] for r in res.results], B, n_cores, T - cf)
    return out.astype(np.float32)

